# revision 1
# baseline (speedup 1.0000x reference)
"""CrossMultiHeadedAttention Trainium2 kernel.

Problem: B=4, S=2048, H=512, NH=8 heads, D=64.
  qh = (q @ Wq + bq), kh = (k @ Wk + bk), kbh = (k_b @ Wkb + bkb), vh = (v @ Wv + bv)
  scores = qh @ (kh + kbh)^T / sqrt(D), masked where mask[key]==0, softmax over keys
  out = (softmax @ vh heads concat) @ Wo + bo

Sharding: 8 cores = 4 batches x 2 head-groups (4 heads each).  Tensor-parallel
on the projections (Wq/Wk/Wv/Wkb column-split, Wo row-split); each core emits a
partial [S, H] output; host sums the two head-group partials per batch + bo.

Sparsity: the mask depends only on the key index, and masked keys contribute
exactly 0 after softmax (exp underflow), so the host gathers only unmasked
k/k_b/v rows (padded to a multiple of 128).  Padded keys are killed with a
-1e9 additive bias before exp.

Device layouts (per core):
  x^T tiles [H, *] via PE transpose  ->  Q^T [64, S] / K'^T [64, npad] per head
  scores^T [keys, queries]:  softmax normalizer via a ones-column appended to V
  (PV matmul emits [65, q]: row 64 = sum of exp), mask bias is per-partition.
  O' normalized by 1/l via gpsimd partition-broadcast + DVE multiply, then the
  output projection contracts the 4 heads' dims in PSUM.
"""

import math

import numpy as np

import concourse.bass as bass
import concourse.tile as tile
from concourse import mybir
from concourse.masks import make_identity

F32 = mybir.dt.float32
F32R = mybir.dt.float32r

B, S, H, NH = 4, 2048, 512, 8
D = H // NH          # 64
HG = 4               # heads per core
HS = HG * D          # 256, per-core projection width
NEG = -1.0e9


def r(ap):
    """Use the fast fp32r PE path for matmul operands."""
    return ap.bitcast(F32R)


def build_nc(npad: int) -> bass.Bass:
    KT = npad // 128          # key tiles
    QC = S // 512             # query chunks of 512
    nc = bass.Bass(target_bir_lowering=False, debug=False)

    xq = nc.declare_dram_parameter("xq", [S, H], F32, isOutput=False)[:]
    xk = nc.declare_dram_parameter("xk", [npad, H], F32, isOutput=False)[:]
    xkb = nc.declare_dram_parameter("xkb", [npad, H], F32, isOutput=False)[:]
    xv = nc.declare_dram_parameter("xv", [npad, H], F32, isOutput=False)[:]
    mb = nc.declare_dram_parameter("mb", [npad], F32, isOutput=False)[:]
    wq = nc.declare_dram_parameter("wq", [H, HS], F32R, isOutput=False)[:]
    wk = nc.declare_dram_parameter("wk", [H, HS], F32R, isOutput=False)[:]
    wkb = nc.declare_dram_parameter("wkb", [H, HS], F32R, isOutput=False)[:]
    wv = nc.declare_dram_parameter("wv", [640, HS], F32R, isOutput=False)[:]
    wo = nc.declare_dram_parameter("wo", [HS, H], F32R, isOutput=False)[:]
    bq = nc.declare_dram_parameter("bq", [HS], F32, isOutput=False)[:]
    bkk = nc.declare_dram_parameter("bkk", [HS], F32, isOutput=False)[:]
    out = nc.declare_dram_parameter("out", [S, H], F32, isOutput=True)[:]
    lscratch = nc.dram_tensor("lscratch", [S // 512 * HG, 512], F32)[:]

    Identity = mybir.ActivationFunctionType.Identity
    Exp = mybir.ActivationFunctionType.Exp

    with tile.TileContext(nc) as tc:
        with (
            tc.tile_pool(name="const", bufs=1) as cpool,
            tc.tile_pool(name="persist", bufs=1) as ppool,
            tc.tile_pool(name="stage", bufs=8) as stage,
            tc.tile_pool(name="xtc", bufs=4) as xtpool,
            tc.tile_pool(name="probs", bufs=4) as prpool,
            tc.tile_pool(name="norm", bufs=3) as nrpool,
            tc.tile_pool(name="outs", bufs=3) as outpool,
        ):
            ident = cpool.tile([128, 128], F32)
            make_identity(nc, ident[:])
            wq_sb = cpool.tile([128, 4, HS], F32R, tag="wq")
            nc.sync.dma_start(wq_sb[:], wq.rearrange("(t p) n -> p t n", p=128))
            wk_sb = cpool.tile([128, 4, HS], F32R, tag="wk")
            nc.sync.dma_start(wk_sb[:], wk.rearrange("(t p) n -> p t n", p=128))
            wkb_sb = cpool.tile([128, 4, HS], F32R, tag="wkb")
            nc.sync.dma_start(wkb_sb[:], wkb.rearrange("(t p) n -> p t n", p=128))
            wv_sb = cpool.tile([128, 5, HS], F32R, tag="wv")
            nc.sync.dma_start(wv_sb[:], wv.rearrange("(t p) n -> p t n", p=128))
            wo_sb = cpool.tile([128, 2, H], F32R, tag="wo")
            nc.sync.dma_start(wo_sb[:], wo.rearrange("(t p) n -> p t n", p=128))
            bq_sb = cpool.tile([128, 2], F32, tag="bq")
            nc.sync.dma_start(bq_sb[:], bq.rearrange("(t p) -> p t", p=128))
            bkk_sb = cpool.tile([128, 2], F32, tag="bkk")
            nc.sync.dma_start(bkk_sb[:], bkk.rearrange("(t p) -> p t", p=128))
            mb_sb = cpool.tile([128, KT], F32, tag="mb")
            nc.sync.dma_start(mb_sb[:], mb.rearrange("(t p) -> p t", p=128))

            ones_c = cpool.tile([128, 512], F32, tag="ones")
            nc.gpsimd.memset(ones_c[:], 1.0)

            qT = ppool.tile([128, 2, S], F32, tag="qT")
            kT = ppool.tile([128, 2, npad], F32, tag="kT")
            v_sb = ppool.tile([128, KT, HG, 65], F32, tag="v")
            o_sb = ppool.tile([128, 2, S], F32, tag="o")
            nc.vector.tensor_copy(
                r(v_sb[:, :, :, 64:65]),
                ones_c[:, :KT * HG].rearrange("p (k h o) -> p k h o", k=KT, o=1),
            )

            with tc.tile_pool(name="ps_a", bufs=4, space="PSUM") as ps_a:
                # ---- Q path: transpose + project, streamed per 512 tokens ----
                for c in range(QC):
                    sts = []
                    for i in range(4):
                        st = stage.tile([128, H], F32, tag="stage")
                        nc.sync.dma_start(st[:], xq[c * 512 + i * 128:c * 512 + (i + 1) * 128, :])
                        sts.append(st)
                    xtc = xtpool.tile([128, 4, 512], F32, tag="xtc")
                    for t in range(4):
                        pst = ps_a.tile([128, 512], F32, tag="ps_a")
                        for i in range(4):
                            nc.tensor.transpose(
                                pst[:, i * 128:(i + 1) * 128],
                                sts[i][:, t * 128:(t + 1) * 128],
                                ident[:],
                            )
                        nc.vector.tensor_copy(r(xtc[:, t, :]), pst[:])
                    for hp in range(2):
                        psq = ps_a.tile([128, 512], F32, tag="ps_a")
                        for t in range(4):
                            nc.tensor.matmul(
                                psq[:],
                                r(wq_sb[:, t, hp * 128:(hp + 1) * 128]),
                                r(xtc[:, t, :]),
                                start=(t == 0),
                                stop=(t == 3),
                            )
                        nc.scalar.activation(
                            r(qT[:, hp, c * 512:(c + 1) * 512]), psq[:],
                            Identity, bias=bq_sb[:, hp:hp + 1],
                        )

                # ---- K'/V path, streamed per key chunk (<=512 keys) ----
                kcw = []
                off = 0
                while off < npad:
                    w = min(512, npad - off)
                    kcw.append((off, w))
                    off += w
                for (off, w) in kcw:
                    nsub = w // 128
                    stk, stkb, stv = [], [], []
                    for i in range(nsub):
                        a = stage.tile([128, H], F32, tag="stage")
                        nc.sync.dma_start(a[:], xk[off + i * 128:off + (i + 1) * 128, :])
                        stk.append(a)
                        b_ = stage.tile([128, H], F32, tag="stage")
                        nc.sync.dma_start(b_[:], xkb[off + i * 128:off + (i + 1) * 128, :])
                        stkb.append(b_)
                        c_ = stage.tile([128, H], F32, tag="stage")
                        nc.sync.dma_start(c_[:], xv[off + i * 128:off + (i + 1) * 128, :])
                        stv.append(c_)
                    xk_t = xtpool.tile([128, 4, 512], F32, tag="xtc")
                    xkb_t = xtpool.tile([128, 4, 512], F32, tag="xtc")
                    xv_t = xtpool.tile([128, 5, 512], F32, tag="xtcv")
                    for (src, dst) in ((stk, xk_t), (stkb, xkb_t), (stv, xv_t)):
                        for t in range(4):
                            pst = ps_a.tile([128, 512], F32, tag="ps_a")
                            for i in range(nsub):
                                nc.tensor.transpose(
                                    pst[:, i * 128:(i + 1) * 128],
                                    src[i][:, t * 128:(t + 1) * 128],
                                    ident[:],
                                )
                            nc.vector.tensor_copy(r(dst[:, t, :w]), pst[:, :w])
                    nc.vector.tensor_copy(r(xv_t[0:1, 4, :w]), ones_c[0:1, :w])
                    for hp in range(2):
                        psk = ps_a.tile([128, 512], F32, tag="ps_a")
                        for t in range(4):
                            nc.tensor.matmul(
                                psk[:, :w],
                                r(wk_sb[:, t, hp * 128:(hp + 1) * 128]),
                                r(xk_t[:, t, :w]),
                                start=(t == 0), stop=False,
                            )
                        for t in range(4):
                            nc.tensor.matmul(
                                psk[:, :w],
                                r(wkb_sb[:, t, hp * 128:(hp + 1) * 128]),
                                r(xkb_t[:, t, :w]),
                                start=False, stop=(t == 3),
                            )
                        nc.scalar.activation(
                            r(kT[:, hp, off:off + w]), psk[:, :w],
                            Identity, bias=bkk_sb[:, hp:hp + 1],
                        )
                    for i in range(nsub):
                        kt_g = off // 128 + i
                        psv = ps_a.tile([128, 512], F32, tag="ps_a")
                        for t in range(5):
                            lhs = xv_t[0:1, 4, i * 128:(i + 1) * 128] if t == 4 \
                                else xv_t[:, t, i * 128:(i + 1) * 128]
                            rhs = wv_sb[0:1, 4, :] if t == 4 else wv_sb[:, t, :]
                            nc.tensor.matmul(
                                psv[:, :HS], r(lhs), r(rhs),
                                start=(t == 0), stop=(t == 4),
                            )
                        nc.vector.tensor_copy(
                            r(v_sb[:, kt_g, :, 0:64]),
                            psv[:, :HS].rearrange("p (h d) -> p h d", h=HG),
                        )

            # ---- attention + output projection ----
            with (
                tc.tile_pool(name="ps_s", bufs=2, space="PSUM") as ps_s,
                tc.tile_pool(name="ps_o", bufs=2, space="PSUM") as ps_o,
                tc.tile_pool(name="ps_f", bufs=2, space="PSUM") as ps_f,
            ):
                for c in range(QC):
                    for h in range(HG):
                        hp, hd = h // 2, h % 2
                        dlo, dhi = hd * 64, (hd + 1) * 64
                        pso = ps_o.tile([65, 512], F32, tag="ps_o")
                        for kt in range(KT):
                            pss = ps_s.tile([128, 512], F32, tag="ps_s")
                            nc.tensor.matmul(
                                pss[:],
                                r(kT[dlo:dhi, hp, kt * 128:(kt + 1) * 128]),
                                r(qT[dlo:dhi, hp, c * 512:(c + 1) * 512]),
                                start=True, stop=True,
                            )
                            p = prpool.tile([128, 512], F32, tag="p")
                            nc.scalar.activation(r(p[:]), pss[:], Exp, bias=mb_sb[:, kt:kt + 1])
                            nc.tensor.matmul(
                                pso[:], r(v_sb[:, kt, h, :]), r(p[:]),
                                start=(kt == 0), stop=(kt == KT - 1),
                            )
                        linv = nrpool.tile([1, 512], F32, tag="linv")
                        nc.vector.reciprocal(linv[:], pso[64:65, :])
                        lrow = lscratch[c * HG + h:c * HG + h + 1, :]
                        nc.sync.dma_start(lrow, linv[:])
                        lbc = nrpool.tile([64, 512], F32, tag="lbc")
                        lsrc, _ = bass.broadcast_tensor_aps(lrow, lbc[:])
                        nc.sync.dma_start(lbc[:], lsrc)
                        nc.vector.tensor_mul(
                            r(o_sb[dlo:dhi, hp, c * 512:(c + 1) * 512]),
                            pso[0:64, :], lbc[:],
                        )
                    for sidx in range(4):
                        tt = c * 4 + sidx
                        psf = ps_f.tile([128, 512], F32, tag="ps_f")
                        for hp in range(2):
                            nc.tensor.matmul(
                                psf[:],
                                r(o_sb[:, hp, tt * 128:(tt + 1) * 128]),
                                r(wo_sb[:, hp, :]),
                                start=(hp == 0), stop=(hp == 1),
                            )
                        ob = outpool.tile([128, H], F32, tag="ob")
                        nc.vector.tensor_copy(ob[:], psf[:])
                        nc.sync.dma_start(out[tt * 128:(tt + 1) * 128, :], ob[:])
    _split_matmul_waits(nc)
    return nc


def _split_matmul_waits(nc: bass.Bass):
    """Walrus's fp32r matmul (LDW+MM) and DMA lowerings only fit one sync
    wait, but Tile may attach several.  Move the extras onto same-queue NOPs
    inserted right before each offending instruction."""
    eng_map = {
        mybir.EngineType.PE: nc.tensor,
        mybir.EngineType.SP: nc.sync,
        mybir.EngineType.Activation: nc.scalar,
        mybir.EngineType.DVE: nc.vector,
        mybir.EngineType.Pool: nc.gpsimd,
    }
    f = nc.m.functions[0]
    blocks = list(f.blocks)
    endblk = blocks[-1]
    n_nops = 0
    for blk in blocks:
        insts = blk.instructions
        if not any(
                x.sync_info is not None and len(x.sync_info.on_wait) > 1
                for x in insts):
            continue
        new = []
        changed = False
        for inst in insts:
            si = inst.sync_info
            if (si is not None and len(si.on_wait) > 1
                    and inst.engine in eng_map):
                waits = list(si.on_wait)
                for w in waits[:-1]:
                    nop = eng_map[inst.engine].nop().ins
                    n_nops += 1
                    nop.sync_info = type(si)(on_wait=[w], on_update=[])
                    new.append(nop)
                inst.sync_info = type(si)(on_wait=[waits[-1]],
                                          on_update=list(si.on_update))
                changed = True
            new.append(inst)
        if changed:
            blk.instructions = new
    if n_nops:
        # the .nop() calls appended to the tail block; strip them.
        endblk.instructions = endblk.instructions[:-n_nops]


_NC_CACHE: dict[int, bass.Bass] = {}


def _get_nc(npad: int) -> bass.Bass:
    if npad not in _NC_CACHE:
        _NC_CACHE[npad] = build_nc(npad)
    return _NC_CACHE[npad]


def make_in_maps(q, k, v, k_b, mask, Wq, bq, Wk, bk, Wv, bv, Wkb, bkb, Wo, bo):
    """Host-side sharding: returns (in_maps for cores 0..7, npad)."""
    f = np.float32
    sels = [np.nonzero(mask[b])[0] for b in range(B)]
    nmax = max(len(s) for s in sels)
    npad = max(128, int(math.ceil(nmax / 128.0)) * 128)
    scale = f(1.0 / math.sqrt(D))

    batch_data = []
    for b in range(B):
        sel = sels[b]
        n = len(sel)
        xk_g = np.zeros((npad, H), f)
        xkb_g = np.zeros((npad, H), f)
        xv_g = np.zeros((npad, H), f)
        xk_g[:n] = k[b][sel]
        xkb_g[:n] = k_b[b][sel]
        xv_g[:n] = v[b][sel]
        mb = np.full((npad,), NEG, f)
        mb[:n] = 0.0
        batch_data.append((np.ascontiguousarray(q[b], f), xk_g, xkb_g, xv_g, mb))

    group_data = []
    for hg in range(2):
        cs = slice(hg * HS, (hg + 1) * HS)
        wv_aug = np.zeros((640, HS), f)
        wv_aug[:H] = Wv[:, cs]
        wv_aug[H] = bv[cs]
        group_data.append(dict(
            wq=np.ascontiguousarray(Wq[:, cs] * scale, f),
            wk=np.ascontiguousarray(Wk[:, cs], f),
            wkb=np.ascontiguousarray(Wkb[:, cs], f),
            wv=wv_aug,
            wo=np.ascontiguousarray(Wo[cs, :], f),
            bq=np.ascontiguousarray(bq[cs] * scale, f),
            bkk=np.ascontiguousarray((bk + bkb)[cs], f),
        ))

    in_maps = []
    for core in range(8):
        b, hg = core // 2, core % 2
        xq_b, xk_g, xkb_g, xv_g, mb = batch_data[b]
        m = dict(xq=xq_b, xk=xk_g, xkb=xkb_g, xv=xv_g, mb=mb)
        m.update(group_data[hg])
        in_maps.append(m)
    return in_maps, npad


def kernel(q, k, v, k_b, mask, Wq, bq, Wk, bk, Wv, bv, Wkb, bkb, Wo, bo):
    from concourse.bass_utils import run_bass_kernel_spmd

    q, k, v, k_b = (np.asarray(x, np.float32) for x in (q, k, v, k_b))
    mask = np.asarray(mask)
    in_maps, npad = make_in_maps(q, k, v, k_b, mask, Wq, bq, Wk, bk, Wv, bv,
                                 Wkb, bkb, Wo, bo)
    nc = _get_nc(npad)
    res = run_bass_kernel_spmd(nc, in_maps, list(range(8))).results
    bo = np.asarray(bo, np.float32)
    out = np.empty((B, S, H), np.float32)
    for b in range(B):
        out[b] = res[2 * b]["out"] + res[2 * b + 1]["out"] + bo
    return out



# revision 2
# speedup vs baseline: 1.2143x; 1.2143x over previous
"""CrossMultiHeadedAttention Trainium2 kernel (bf16 fast path).

Problem: B=4, S=2048, H=512, NH=8 heads, D=64.
  qh = (q @ Wq + bq), kh = (k @ Wk + bk), kbh = (k_b @ Wkb + bkb), vh = (v @ Wv + bv)
  scores = qh @ (kh + kbh)^T / sqrt(D), masked where mask[key]==0, softmax over keys
  out = (softmax @ vh heads concat) @ Wo + bo

Sharding: 8 cores = 4 batches x 2 head-groups (4 heads each).  Tensor-parallel
on the projections (Wq/Wk/Wv/Wkb column-split, Wo row-split); each core emits a
partial [S, H] output; host sums the two head-group partials per batch + bo.

Host-side prep (data movement only): gather unmasked k/k_b/v rows (mask depends
only on the key index; masked keys contribute exactly 0 after softmax), pad to
a multiple of 128, pre-transpose activations to [H, *] layout, and cast to
bf16.  This removes all PE transposes from the device program and keeps the PE
on the unthrottled bf16 path (fp32r matmuls are power-throttled to ~50% duty).

Padded keys carry zero k'/v rows.  Instead of a -1e9 additive bias before exp,
the softmax-denominator ones-column appended to V holds the key-validity mask
(1 real / 0 pad) and the V rows are scaled by it: pads then contribute exactly
0 to both the numerator and denominator, so exp needs no bias at all and can
drain two score PSUM banks per activation instruction.

Device layouts (per core):
  scores^T [keys, queries] per head; softmax normalizer via the validity
  column appended to V (PV matmul emits [65, q]: row 64 = sum of exp over real
  keys).  O' is normalized by 1/l via a DRAM-broadcast + DVE multiply, then
  the output projection contracts the 4 heads' dims in PSUM.
"""

import math

import ml_dtypes
import numpy as np

import concourse.bass as bass
import concourse.tile as tile
from concourse import mybir

F32 = mybir.dt.float32
BF16 = mybir.dt.bfloat16
NPBF16 = ml_dtypes.bfloat16

B, S, H, NH = 4, 2048, 512, 8
D = H // NH          # 64
HG = 4               # heads per core
HS = HG * D          # 256, per-core projection width
QC = S // 512        # query chunks of 512


def build_nc(npad: int) -> bass.Bass:
    KT = npad // 128          # key tiles
    nc = bass.Bass(target_bir_lowering=False, debug=False)

    xq = nc.declare_dram_parameter("xq", [H, S], BF16, isOutput=False)[:]
    xk = nc.declare_dram_parameter("xk", [H, npad], BF16, isOutput=False)[:]
    xkb = nc.declare_dram_parameter("xkb", [H, npad], BF16, isOutput=False)[:]
    xv = nc.declare_dram_parameter("xv", [H, npad], BF16, isOutput=False)[:]
    vm = nc.declare_dram_parameter("vm", [npad], F32, isOutput=False)[:]
    wq = nc.declare_dram_parameter("wq", [H, HS], BF16, isOutput=False)[:]
    wk = nc.declare_dram_parameter("wk", [H, HS], BF16, isOutput=False)[:]
    wkb = nc.declare_dram_parameter("wkb", [H, HS], BF16, isOutput=False)[:]
    wv = nc.declare_dram_parameter("wv", [640, HS], BF16, isOutput=False)[:]
    wo = nc.declare_dram_parameter("wo", [HS, H], BF16, isOutput=False)[:]
    bq = nc.declare_dram_parameter("bq", [HS], F32, isOutput=False)[:]
    bkk = nc.declare_dram_parameter("bkk", [HS], F32, isOutput=False)[:]
    out = nc.declare_dram_parameter("out", [S, H], F32, isOutput=True)[:]
    lscratch = nc.dram_tensor("lscratch", [QC * HG, 512], F32)[:]

    Exp = mybir.ActivationFunctionType.Exp

    # key chunks of <=512 for the K' projection / DMA staging
    kcw = []
    off = 0
    while off < npad:
        w = min(512, npad - off)
        kcw.append((off, w))
        off += w
    # kt tile pairs for two-bank exp drains
    groups = [list(range(g, min(g + 2, KT))) for g in range(0, KT, 2)]

    with tile.TileContext(nc) as tc:
        with (
            tc.tile_pool(name="const", bufs=1) as cpool,
            tc.tile_pool(name="persist", bufs=1) as ppool,
            tc.tile_pool(name="probs", bufs=4) as prpool,
            tc.tile_pool(name="norm", bufs=3) as nrpool,
            tc.tile_pool(name="outs", bufs=3) as outpool,
        ):
            wq_sb = cpool.tile([128, 4, HS], BF16, tag="wq")
            nc.sync.dma_start(wq_sb[:], wq.rearrange("(t p) n -> p t n", p=128))
            wk_sb = cpool.tile([128, 4, HS], BF16, tag="wk")
            nc.sync.dma_start(wk_sb[:], wk.rearrange("(t p) n -> p t n", p=128))
            wkb_sb = cpool.tile([128, 4, HS], BF16, tag="wkb")
            nc.sync.dma_start(wkb_sb[:], wkb.rearrange("(t p) n -> p t n", p=128))
            wv_sb = cpool.tile([128, 5, HS], BF16, tag="wv")
            nc.sync.dma_start(wv_sb[:], wv.rearrange("(t p) n -> p t n", p=128))
            wo_sb = cpool.tile([128, 2, H], BF16, tag="wo")
            nc.sync.dma_start(wo_sb[:], wo.rearrange("(t p) n -> p t n", p=128))
            bq_sb = cpool.tile([128, 2], F32, tag="bq")
            nc.sync.dma_start(bq_sb[:], bq.rearrange("(t p) -> p t", p=128))
            bkk_sb = cpool.tile([128, 2], F32, tag="bkk")
            nc.sync.dma_start(bkk_sb[:], bkk.rearrange("(t p) -> p t", p=128))
            vm_sb = cpool.tile([128, KT], F32, tag="vm")
            nc.sync.dma_start(vm_sb[:], vm.rearrange("(t p) -> p t", p=128))
            ones_c = cpool.tile([1, 128], BF16, tag="ones")
            nc.gpsimd.memset(ones_c[:], 1.0)

            # input activations, pre-transposed [H, *] -> [128, 4, *]
            kx = ppool.tile([128, 4, npad], BF16, tag="kx")
            kbx = ppool.tile([128, 4, npad], BF16, tag="kbx")
            vx = ppool.tile([128, 4, npad], BF16, tag="vx")
            qx = ppool.tile([128, 4, S], BF16, tag="qx")
            xk_r = xk.rearrange("(t p) n -> p t n", p=128)
            xkb_r = xkb.rearrange("(t p) n -> p t n", p=128)
            xv_r = xv.rearrange("(t p) n -> p t n", p=128)
            xq_r = xq.rearrange("(t p) n -> p t n", p=128)
            for (off, w) in kcw:
                for t in range(4):
                    nc.sync.dma_start(kx[:, t, off:off + w], xk_r[:, t, off:off + w])
                    nc.sync.dma_start(kbx[:, t, off:off + w], xkb_r[:, t, off:off + w])
            for (off, w) in kcw:
                for t in range(4):
                    nc.sync.dma_start(vx[:, t, off:off + w], xv_r[:, t, off:off + w])
            for c in range(QC):
                for t in range(4):
                    nc.sync.dma_start(
                        qx[:, t, c * 512:(c + 1) * 512],
                        xq_r[:, t, c * 512:(c + 1) * 512],
                    )

            qT = ppool.tile([128, 2, S], BF16, tag="qT")
            kT = ppool.tile([128, 2, npad], BF16, tag="kT")
            v_sb = ppool.tile([128, KT, HG, 65], BF16, tag="v")
            o_sb = ppool.tile([128, 2, S], BF16, tag="o")
            # validity column (softmax denominator counts real keys only)
            vm_col = vm_sb[:].rearrange("p (t o) -> p t o", o=1)
            for h in range(HG):
                nc.gpsimd.tensor_copy(v_sb[:, :, h, 64:65], vm_col)

            with (
                tc.tile_pool(name="ps_p", bufs=2, space="PSUM") as ps_p,
                tc.tile_pool(name="ps_s", bufs=2, space="PSUM") as ps_s,
                tc.tile_pool(name="ps_o", bufs=2, space="PSUM") as ps_o,
            ):
                # ---- K' = k@Wk + k_b@Wkb + (bk+bkb), transposed layout ----
                for (off, w) in kcw:
                    for hp in range(2):
                        psk = ps_p.tile([128, 512], F32, tag="ps_p")
                        for t in range(4):
                            nc.tensor.matmul(
                                psk[:, :w],
                                wk_sb[:, t, hp * 128:(hp + 1) * 128],
                                kx[:, t, off:off + w],
                                start=(t == 0), stop=False,
                            )
                        for t in range(4):
                            nc.tensor.matmul(
                                psk[:, :w],
                                wkb_sb[:, t, hp * 128:(hp + 1) * 128],
                                kbx[:, t, off:off + w],
                                start=False, stop=(t == 3),
                            )
                        nc.vector.tensor_scalar_add(
                            kT[:, hp, off:off + w], psk[:, :w], bkk_sb[:, hp:hp + 1]
                        )

                # ---- V rows (gated by validity so pads contribute 0) ----
                for kt in range(KT):
                    psv = ps_p.tile([128, 512], F32, tag="ps_p")
                    for t in range(4):
                        nc.tensor.matmul(
                            psv[:, :HS],
                            vx[:, t, kt * 128:(kt + 1) * 128],
                            wv_sb[:, t, :],
                            start=(t == 0), stop=False,
                        )
                    nc.tensor.matmul(
                        psv[:, :HS], ones_c[0:1, :], wv_sb[0:1, 4, :],
                        start=False, stop=True,
                    )
                    nc.vector.tensor_scalar_mul(
                        v_sb[:, kt, :, 0:64],
                        psv[:, :HS].rearrange("p (h d) -> p h d", h=HG),
                        vm_sb[:, kt:kt + 1],
                    )

                # ---- per query chunk: Q projection, attention, out proj ----
                for c in range(QC):
                    q0 = c * 512
                    for hp in range(2):
                        psq = ps_p.tile([128, 512], F32, tag="ps_p")
                        for t in range(4):
                            nc.tensor.matmul(
                                psq[:],
                                wq_sb[:, t, hp * 128:(hp + 1) * 128],
                                qx[:, t, q0:q0 + 512],
                                start=(t == 0), stop=(t == 3),
                            )
                        nc.vector.tensor_scalar_add(
                            qT[:, hp, q0:q0 + 512], psq[:], bq_sb[:, hp:hp + 1]
                        )

                    for h in range(HG):
                        hp, hd = h // 2, h % 2
                        dlo, dhi = hd * 64, (hd + 1) * 64
                        pso = ps_o.tile([65, 512], F32, tag="ps_o")
                        first_pv = [True]

                        def drain(pss, grp, last):
                            w = len(grp) * 512
                            p = prpool.tile([128, 1024], BF16, tag="p")
                            nc.scalar.activation(p[:, :w], pss[:, :w], Exp)
                            for j, kt in enumerate(grp):
                                nc.tensor.matmul(
                                    pso[:],
                                    v_sb[:, kt, h, :],
                                    p[:, j * 512:(j + 1) * 512],
                                    start=first_pv[0],
                                    stop=(last and j == len(grp) - 1),
                                )
                                first_pv[0] = False

                        prev = None
                        for grp in groups:
                            pss = ps_s.tile([128, 1024], F32, tag="ps_s")
                            for j, kt in enumerate(grp):
                                nc.tensor.matmul(
                                    pss[:, j * 512:(j + 1) * 512],
                                    kT[dlo:dhi, hp, kt * 128:(kt + 1) * 128],
                                    qT[dlo:dhi, hp, q0:q0 + 512],
                                    start=True, stop=True,
                                )
                            if prev is not None:
                                drain(*prev, last=False)
                            prev = (pss, grp)
                        drain(*prev, last=True)

                        linv = nrpool.tile([1, 512], F32, tag="linv")
                        nc.vector.reciprocal(linv[:], pso[64:65, :])
                        lrow = lscratch[c * HG + h:c * HG + h + 1, :]
                        nc.sync.dma_start(lrow, linv[:])
                        lbc = nrpool.tile([64, 512], F32, tag="lbc")
                        lsrc, _ = bass.broadcast_tensor_aps(lrow, lbc[:])
                        nc.sync.dma_start(lbc[:], lsrc)
                        nc.vector.tensor_mul(
                            o_sb[dlo:dhi, hp, q0:q0 + 512], pso[0:64, :], lbc[:]
                        )

                    for sidx in range(4):
                        tt = c * 4 + sidx
                        psf = ps_p.tile([128, 512], F32, tag="ps_p")
                        for hp in range(2):
                            nc.tensor.matmul(
                                psf[:],
                                o_sb[:, hp, tt * 128:(tt + 1) * 128],
                                wo_sb[:, hp, :],
                                start=(hp == 0), stop=(hp == 1),
                            )
                        ob = outpool.tile([128, H], F32, tag="ob")
                        nc.vector.tensor_copy(ob[:], psf[:])
                        nc.sync.dma_start(out[tt * 128:(tt + 1) * 128, :], ob[:])
    _split_matmul_waits(nc)
    return nc


def _split_matmul_waits(nc: bass.Bass):
    """Walrus's matmul (LDW+MM) and DMA lowerings only fit one sync
    wait, but Tile may attach several.  Move the extras onto same-queue NOPs
    inserted right before each offending instruction."""
    eng_map = {
        mybir.EngineType.PE: nc.tensor,
        mybir.EngineType.SP: nc.sync,
        mybir.EngineType.Activation: nc.scalar,
        mybir.EngineType.DVE: nc.vector,
        mybir.EngineType.Pool: nc.gpsimd,
    }
    f = nc.m.functions[0]
    blocks = list(f.blocks)
    endblk = blocks[-1]
    n_nops = 0
    for blk in blocks:
        insts = blk.instructions
        if not any(
                x.sync_info is not None and len(x.sync_info.on_wait) > 1
                for x in insts):
            continue
        new = []
        changed = False
        for inst in insts:
            si = inst.sync_info
            if (si is not None and len(si.on_wait) > 1
                    and inst.engine in eng_map):
                waits = list(si.on_wait)
                for w in waits[:-1]:
                    nop = eng_map[inst.engine].nop().ins
                    n_nops += 1
                    nop.sync_info = type(si)(on_wait=[w], on_update=[])
                    new.append(nop)
                inst.sync_info = type(si)(on_wait=[waits[-1]],
                                          on_update=list(si.on_update))
                changed = True
            new.append(inst)
        if changed:
            blk.instructions = new
    if n_nops:
        # the .nop() calls appended to the tail block; strip them.
        endblk.instructions = endblk.instructions[:-n_nops]


_NC_CACHE: dict[int, bass.Bass] = {}


def _get_nc(npad: int) -> bass.Bass:
    if npad not in _NC_CACHE:
        _NC_CACHE[npad] = build_nc(npad)
    return _NC_CACHE[npad]


def make_in_maps(q, k, v, k_b, mask, Wq, bq, Wk, bk, Wv, bv, Wkb, bkb, Wo, bo):
    """Host-side sharding: returns (in_maps for cores 0..7, npad)."""
    f = np.float32
    bf = NPBF16
    sels = [np.nonzero(mask[b])[0] for b in range(B)]
    nmax = max(len(s) for s in sels)
    npad = max(128, int(math.ceil(nmax / 128.0)) * 128)
    scale = f(1.0 / math.sqrt(D))

    batch_data = []
    for b in range(B):
        sel = sels[b]
        n = len(sel)
        xq_t = np.ascontiguousarray(q[b].T).astype(bf)
        xk_t = np.zeros((H, npad), bf)
        xkb_t = np.zeros((H, npad), bf)
        xv_t = np.zeros((H, npad), bf)
        xk_t[:, :n] = k[b][sel].T.astype(bf)
        xkb_t[:, :n] = k_b[b][sel].T.astype(bf)
        xv_t[:, :n] = v[b][sel].T.astype(bf)
        vm = np.zeros((npad,), f)
        vm[:n] = 1.0
        batch_data.append((xq_t, xk_t, xkb_t, xv_t, vm))

    group_data = []
    for hg in range(2):
        cs = slice(hg * HS, (hg + 1) * HS)
        wv_aug = np.zeros((640, HS), bf)
        wv_aug[:H] = Wv[:, cs].astype(bf)
        wv_aug[H] = bv[cs].astype(bf)
        group_data.append(dict(
            wq=np.ascontiguousarray(Wq[:, cs] * scale).astype(bf),
            wk=np.ascontiguousarray(Wk[:, cs]).astype(bf),
            wkb=np.ascontiguousarray(Wkb[:, cs]).astype(bf),
            wv=wv_aug,
            wo=np.ascontiguousarray(Wo[cs, :]).astype(bf),
            bq=np.ascontiguousarray(bq[cs] * scale, f),
            bkk=np.ascontiguousarray((bk + bkb)[cs], f),
        ))

    in_maps = []
    for core in range(8):
        b, hg = core // 2, core % 2
        xq_t, xk_t, xkb_t, xv_t, vm = batch_data[b]
        m = dict(xq=xq_t, xk=xk_t, xkb=xkb_t, xv=xv_t, vm=vm)
        m.update(group_data[hg])
        in_maps.append(m)
    return in_maps, npad


def kernel(q, k, v, k_b, mask, Wq, bq, Wk, bk, Wv, bv, Wkb, bkb, Wo, bo):
    from concourse.bass_utils import run_bass_kernel_spmd

    q, k, v, k_b = (np.asarray(x, np.float32) for x in (q, k, v, k_b))
    mask = np.asarray(mask)
    in_maps, npad = make_in_maps(q, k, v, k_b, mask, Wq, bq, Wk, bk, Wv, bv,
                                 Wkb, bkb, Wo, bo)
    nc = _get_nc(npad)
    res = run_bass_kernel_spmd(nc, in_maps, list(range(8))).results
    bo = np.asarray(bo, np.float32)
    out = np.empty((B, S, H), np.float32)
    for b in range(B):
        out[b] = res[2 * b]["out"] + res[2 * b + 1]["out"] + bo
    return out


# revision 6
# speedup vs baseline: 1.3237x; 1.0901x over previous
"""CrossMultiHeadedAttention Trainium2 kernel (bf16 fast path).

Problem: B=4, S=2048, H=512, NH=8 heads, D=64.
  qh = (q @ Wq + bq), kh = (k @ Wk + bk), kbh = (k_b @ Wkb + bkb), vh = (v @ Wv + bv)
  scores = qh @ (kh + kbh)^T / sqrt(D), masked where mask[key]==0, softmax over keys
  out = (softmax @ vh heads concat) @ Wo + bo

Sharding: 8 cores = 4 batches x 2 head-groups (4 heads each).  Tensor-parallel
on the projections (Wq/Wk/Wv/Wkb column-split, Wo row-split); each core emits a
partial [S, H] output; host sums the two head-group partials per batch + bo.

Host-side prep (data movement only): gather unmasked k/k_b/v rows (mask depends
only on the key index; masked keys contribute exactly 0 after softmax), pad to
a multiple of 128, pre-transpose activations to [H, *] layout, and cast to
bf16.  This removes all PE transposes from the device program and keeps the PE
on the unthrottled bf16 path (fp32r matmuls are power-throttled to ~50% duty).

Padded keys carry zero k'/v rows.  Instead of a -1e9 additive bias before exp,
the softmax-denominator ones-column appended to V holds the key-validity mask
(1 real / 0 pad) and the V rows are scaled by it: pads then contribute exactly
0 to both the numerator and denominator, so exp needs no bias at all and can
drain two score PSUM banks per activation instruction.

Device layouts (per core):
  scores^T [keys, queries] per head; softmax normalizer via the validity
  column appended to V (PV matmul emits [65, q]: row 64 = sum of exp over real
  keys).  O' is normalized by 1/l via a DRAM-broadcast + DVE multiply, then
  the output projection contracts the 4 heads' dims in PSUM.
"""

import math

import ml_dtypes
import numpy as np

import concourse.bass as bass
import concourse.tile as tile
from concourse import mybir

F32 = mybir.dt.float32
BF16 = mybir.dt.bfloat16
NPBF16 = ml_dtypes.bfloat16

B, S, H, NH = 4, 2048, 512, 8
D = H // NH          # 64
HG = 4               # heads per core
HS = HG * D          # 256, per-core projection width
QC = S // 512        # query chunks of 512


def build_nc(npad: int) -> bass.Bass:
    KT = npad // 128          # key tiles
    nc = bass.Bass(target_bir_lowering=False, debug=False)

    xq = nc.declare_dram_parameter("xq", [H, S], BF16, isOutput=False)[:]
    xk = nc.declare_dram_parameter("xk", [H, npad], BF16, isOutput=False)[:]
    xkb = nc.declare_dram_parameter("xkb", [H, npad], BF16, isOutput=False)[:]
    xv = nc.declare_dram_parameter("xv", [H, npad], BF16, isOutput=False)[:]
    vm = nc.declare_dram_parameter("vm", [npad], F32, isOutput=False)[:]
    wq = nc.declare_dram_parameter("wq", [H, HS], BF16, isOutput=False)[:]
    wk = nc.declare_dram_parameter("wk", [H, HS], BF16, isOutput=False)[:]
    wkb = nc.declare_dram_parameter("wkb", [H, HS], BF16, isOutput=False)[:]
    wv = nc.declare_dram_parameter("wv", [640, HS], BF16, isOutput=False)[:]
    wo = nc.declare_dram_parameter("wo", [HS, H], BF16, isOutput=False)[:]
    bq = nc.declare_dram_parameter("bq", [HS], F32, isOutput=False)[:]
    bkk = nc.declare_dram_parameter("bkk", [HS], F32, isOutput=False)[:]
    out = nc.declare_dram_parameter("out", [S, H], F32, isOutput=True)[:]
    lscratch = nc.dram_tensor("lscratch", [QC * HG, 512], F32)[:]

    Exp = mybir.ActivationFunctionType.Exp

    # key chunks of <=512 for the K' projection / DMA staging
    kcw = []
    off = 0
    while off < npad:
        w = min(512, npad - off)
        kcw.append((off, w))
        off += w
    # kt tile pairs for two-bank exp drains
    groups = [list(range(g, min(g + 2, KT))) for g in range(0, KT, 2)]

    with tile.TileContext(nc) as tc:
        with (
            tc.tile_pool(name="const", bufs=1) as cpool,
            tc.tile_pool(name="persist", bufs=1) as ppool,
            tc.tile_pool(name="probs", bufs=4) as prpool,
            tc.tile_pool(name="norm", bufs=3) as nrpool,
            tc.tile_pool(name="outs", bufs=3) as outpool,
        ):
            # DMA emission order puts the K-projection's dependencies on the
            # first DMA queues so the PE can start within a few us.
            wk_sb = cpool.tile([128, 4, HS], BF16, tag="wk")
            wkb_sb = cpool.tile([128, 4, HS], BF16, tag="wkb")
            wq_sb = cpool.tile([128, 4, HS], BF16, tag="wq")
            wv_sb = cpool.tile([128, 5, HS], BF16, tag="wv")
            wo_sb = cpool.tile([128, 2, H], BF16, tag="wo")
            wk_r = wk.rearrange("(t p) n -> p t n", p=128)
            wkb_r = wkb.rearrange("(t p) n -> p t n", p=128)
            wq_r = wq.rearrange("(t p) n -> p t n", p=128)
            wv_r = wv.rearrange("(t p) n -> p t n", p=128)

            kx = ppool.tile([128, 4, npad], BF16, tag="kx")
            kbx = ppool.tile([128, 4, npad], BF16, tag="kbx")
            vx = ppool.tile([128, 4, npad], BF16, tag="vx")
            qx = ppool.tile([128, 4, S], BF16, tag="qx")
            xk_r = xk.rearrange("(t p) n -> p t n", p=128)
            xkb_r = xkb.rearrange("(t p) n -> p t n", p=128)
            xv_r = xv.rearrange("(t p) n -> p t n", p=128)
            xq_r = xq.rearrange("(t p) n -> p t n", p=128)

            for t in range(4):
                nc.sync.dma_start(wk_sb[:, t], wk_r[:, t])
                nc.sync.dma_start(wkb_sb[:, t], wkb_r[:, t])
            off0, w0 = kcw[0]
            for t in range(4):
                nc.sync.dma_start(kx[:, t, off0:off0 + w0], xk_r[:, t, off0:off0 + w0])
                nc.sync.dma_start(kbx[:, t, off0:off0 + w0], xkb_r[:, t, off0:off0 + w0])
            bq_sb = cpool.tile([128, 2], F32, tag="bq")
            nc.sync.dma_start(bq_sb[:], bq.rearrange("(t p) -> p t", p=128))
            bkk_sb = cpool.tile([128, 2], F32, tag="bkk")
            nc.sync.dma_start(bkk_sb[:], bkk.rearrange("(t p) -> p t", p=128))
            vm_sb = cpool.tile([128, KT], F32, tag="vm")
            nc.sync.dma_start(vm_sb[:], vm.rearrange("(t p) -> p t", p=128))
            ones_c = cpool.tile([1, 128], BF16, tag="ones")
            nc.gpsimd.memset(ones_c[:], 1.0)
            for (off, w) in kcw[1:]:
                for t in range(4):
                    nc.sync.dma_start(kx[:, t, off:off + w], xk_r[:, t, off:off + w])
                    nc.sync.dma_start(kbx[:, t, off:off + w], xkb_r[:, t, off:off + w])
            for t in range(5):
                nc.sync.dma_start(wv_sb[:, t], wv_r[:, t])
            for (off, w) in kcw:
                for t in range(4):
                    nc.sync.dma_start(vx[:, t, off:off + w], xv_r[:, t, off:off + w])
            for t in range(4):
                nc.sync.dma_start(wq_sb[:, t], wq_r[:, t])
            for c in range(QC):
                for t in range(4):
                    nc.sync.dma_start(
                        qx[:, t, c * 512:(c + 1) * 512],
                        xq_r[:, t, c * 512:(c + 1) * 512],
                    )
            nc.sync.dma_start(wo_sb[:], wo.rearrange("(t p) n -> p t n", p=128))

            qT = ppool.tile([128, 2, S], BF16, tag="qT")
            kT = ppool.tile([128, 2, npad], BF16, tag="kT")
            v_sb = ppool.tile([128, KT, HG, 65], BF16, tag="v")
            o_sb = ppool.tile([128, 2, S], BF16, tag="o")
            # validity column (softmax denominator counts real keys only)
            vm_col = vm_sb[:].rearrange("p (t o) -> p t o", o=1)
            for h in range(HG):
                nc.gpsimd.tensor_copy(v_sb[:, :, h, 64:65], vm_col)

            with (
                tc.tile_pool(name="ps_p", bufs=2, space="PSUM") as ps_p,
                tc.tile_pool(name="ps_s", bufs=2, space="PSUM") as ps_s,
                tc.tile_pool(name="ps_o", bufs=2, space="PSUM") as ps_o,
            ):
                # ---- K' = k@Wk + k_b@Wkb + (bk+bkb), transposed layout ----
                for (off, w) in kcw:
                    for hp in range(2):
                        psk = ps_p.tile([128, 512], F32, tag="ps_p")
                        for t in range(4):
                            nc.tensor.matmul(
                                psk[:, :w],
                                wk_sb[:, t, hp * 128:(hp + 1) * 128],
                                kx[:, t, off:off + w],
                                start=(t == 0), stop=False,
                            )
                        for t in range(4):
                            nc.tensor.matmul(
                                psk[:, :w],
                                wkb_sb[:, t, hp * 128:(hp + 1) * 128],
                                kbx[:, t, off:off + w],
                                start=False, stop=(t == 3),
                            )
                        nc.vector.tensor_scalar_add(
                            kT[:, hp, off:off + w], psk[:, :w], bkk_sb[:, hp:hp + 1]
                        )

                # ---- V rows (gated by validity so pads contribute 0) ----
                for kt in range(KT):
                    psv = ps_p.tile([128, 512], F32, tag="ps_p")
                    for t in range(4):
                        nc.tensor.matmul(
                            psv[:, :HS],
                            vx[:, t, kt * 128:(kt + 1) * 128],
                            wv_sb[:, t, :],
                            start=(t == 0), stop=False,
                        )
                    nc.tensor.matmul(
                        psv[:, :HS], ones_c[0:1, :], wv_sb[0:1, 4, :],
                        start=False, stop=True,
                    )
                    nc.vector.tensor_scalar_mul(
                        v_sb[:, kt, :, 0:64],
                        psv[:, :HS].rearrange("p (h d) -> p h d", h=HG),
                        vm_sb[:, kt:kt + 1],
                    )

                # ---- per query chunk: Q projection, attention, out proj.
                # Qproj(c+1) and outproj(c-1) are emitted between heads of
                # attention(c) as PE filler for the exp-latency gaps. ----
                def qproj(c, hp):
                    q0 = c * 512
                    psq = ps_p.tile([128, 512], F32, tag="ps_p")
                    for t in range(4):
                        nc.tensor.matmul(
                            psq[:],
                            wq_sb[:, t, hp * 128:(hp + 1) * 128],
                            qx[:, t, q0:q0 + 512],
                            start=(t == 0), stop=(t == 3),
                        )
                    nc.vector.tensor_scalar_add(
                        qT[:, hp, q0:q0 + 512], psq[:], bq_sb[:, hp:hp + 1]
                    )

                def outproj(tt):
                    psf = ps_p.tile([128, 512], F32, tag="ps_p")
                    for hp in range(2):
                        nc.tensor.matmul(
                            psf[:],
                            o_sb[:, hp, tt * 128:(tt + 1) * 128],
                            wo_sb[:, hp, :],
                            start=(hp == 0), stop=(hp == 1),
                        )
                    ob = outpool.tile([128, H], F32, tag="ob")
                    nc.vector.tensor_copy(ob[:], psf[:])
                    nc.sync.dma_start(out[tt * 128:tt * 128 + 64, :], ob[0:64, :])
                    nc.sync.dma_start(out[tt * 128 + 64:(tt + 1) * 128, :], ob[64:128, :])

                qproj(0, 0)
                qproj(0, 1)
                for c in range(QC):
                    q0 = c * 512
                    fillers = []
                    if c + 1 < QC:
                        fillers += [lambda hp=hp: qproj(c + 1, hp) for hp in range(2)]
                    if c > 0:
                        fillers += [
                            lambda s=s: [outproj((c - 1) * 4 + 2 * s + i) for i in range(2)]
                            for s in range(2)
                        ]

                    for h in range(HG):
                        hp, hd = h // 2, h % 2
                        dlo, dhi = hd * 64, (hd + 1) * 64
                        pso = ps_o.tile([65, 512], F32, tag="ps_o")
                        first_pv = [True]

                        def drain(pss, grp, last):
                            w = len(grp) * 512
                            p = prpool.tile([128, 1024], BF16, tag="p")
                            nc.scalar.activation(p[:, :w], pss[:, :w], Exp)
                            for j, kt in enumerate(grp):
                                nc.tensor.matmul(
                                    pso[:],
                                    v_sb[:, kt, h, :],
                                    p[:, j * 512:(j + 1) * 512],
                                    start=first_pv[0],
                                    stop=(last and j == len(grp) - 1),
                                )
                                first_pv[0] = False

                        prev = None
                        for grp in groups:
                            pss = ps_s.tile([128, 1024], F32, tag="ps_s")
                            for j, kt in enumerate(grp):
                                nc.tensor.matmul(
                                    pss[:, j * 512:(j + 1) * 512],
                                    kT[dlo:dhi, hp, kt * 128:(kt + 1) * 128],
                                    qT[dlo:dhi, hp, q0:q0 + 512],
                                    start=True, stop=True,
                                )
                            if prev is not None:
                                drain(*prev, last=False)
                            prev = (pss, grp)
                        drain(*prev, last=True)

                        # normalizer: broadcast l via DRAM, reciprocal on 64
                        # lanes (a [1,512] reciprocal would be single-lane)
                        lsb = nrpool.tile([1, 512], F32, tag="lsb")
                        nc.vector.tensor_copy(lsb[:], pso[64:65, :])
                        lrow = lscratch[c * HG + h:c * HG + h + 1, :]
                        nc.sync.dma_start(lrow, lsb[:])
                        lbc = nrpool.tile([64, 512], F32, tag="lbc")
                        lsrc, _ = bass.broadcast_tensor_aps(lrow, lbc[:])
                        nc.sync.dma_start(lbc[:], lsrc)
                        linv = nrpool.tile([64, 512], F32, tag="linv")
                        nc.vector.reciprocal(linv[:], lbc[:])
                        nc.vector.tensor_mul(
                            o_sb[dlo:dhi, hp, q0:q0 + 512], pso[0:64, :], linv[:]
                        )
                        if h < len(fillers):
                            fillers[h]()

                for tt in range((QC - 1) * 4, QC * 4):
                    outproj(tt)
    _split_matmul_waits(nc)
    return nc


def _split_matmul_waits(nc: bass.Bass):
    """Walrus's matmul (LDW+MM) and DMA lowerings only fit one sync
    wait, but Tile may attach several.  Move the extras onto same-queue NOPs
    inserted right before each offending instruction."""
    eng_map = {
        mybir.EngineType.PE: nc.tensor,
        mybir.EngineType.SP: nc.sync,
        mybir.EngineType.Activation: nc.scalar,
        mybir.EngineType.DVE: nc.vector,
        mybir.EngineType.Pool: nc.gpsimd,
    }
    f = nc.m.functions[0]
    blocks = list(f.blocks)
    endblk = blocks[-1]
    n_nops = 0
    for blk in blocks:
        insts = blk.instructions
        if not any(
                x.sync_info is not None and len(x.sync_info.on_wait) > 1
                for x in insts):
            continue
        new = []
        changed = False
        for inst in insts:
            si = inst.sync_info
            if (si is not None and len(si.on_wait) > 1
                    and inst.engine in eng_map):
                waits = list(si.on_wait)
                for w in waits[:-1]:
                    nop = eng_map[inst.engine].nop().ins
                    n_nops += 1
                    nop.sync_info = type(si)(on_wait=[w], on_update=[])
                    new.append(nop)
                inst.sync_info = type(si)(on_wait=[waits[-1]],
                                          on_update=list(si.on_update))
                changed = True
            new.append(inst)
        if changed:
            blk.instructions = new
    if n_nops:
        # the .nop() calls appended to the tail block; strip them.
        endblk.instructions = endblk.instructions[:-n_nops]


_NC_CACHE: dict[int, bass.Bass] = {}


def _get_nc(npad: int) -> bass.Bass:
    if npad not in _NC_CACHE:
        _NC_CACHE[npad] = build_nc(npad)
    return _NC_CACHE[npad]


def make_in_maps(q, k, v, k_b, mask, Wq, bq, Wk, bk, Wv, bv, Wkb, bkb, Wo, bo):
    """Host-side sharding: returns (in_maps for cores 0..7, npad)."""
    f = np.float32
    bf = NPBF16
    sels = [np.nonzero(mask[b])[0] for b in range(B)]
    nmax = max(len(s) for s in sels)
    npad = max(128, int(math.ceil(nmax / 128.0)) * 128)
    scale = f(1.0 / math.sqrt(D))

    batch_data = []
    for b in range(B):
        sel = sels[b]
        n = len(sel)
        xq_t = np.ascontiguousarray(q[b].T).astype(bf)
        xk_t = np.zeros((H, npad), bf)
        xkb_t = np.zeros((H, npad), bf)
        xv_t = np.zeros((H, npad), bf)
        xk_t[:, :n] = k[b][sel].T.astype(bf)
        xkb_t[:, :n] = k_b[b][sel].T.astype(bf)
        xv_t[:, :n] = v[b][sel].T.astype(bf)
        vm = np.zeros((npad,), f)
        vm[:n] = 1.0
        batch_data.append((xq_t, xk_t, xkb_t, xv_t, vm))

    group_data = []
    for hg in range(2):
        cs = slice(hg * HS, (hg + 1) * HS)
        wv_aug = np.zeros((640, HS), bf)
        wv_aug[:H] = Wv[:, cs].astype(bf)
        wv_aug[H] = bv[cs].astype(bf)
        group_data.append(dict(
            wq=np.ascontiguousarray(Wq[:, cs] * scale).astype(bf),
            wk=np.ascontiguousarray(Wk[:, cs]).astype(bf),
            wkb=np.ascontiguousarray(Wkb[:, cs]).astype(bf),
            wv=wv_aug,
            wo=np.ascontiguousarray(Wo[cs, :]).astype(bf),
            bq=np.ascontiguousarray(bq[cs] * scale, f),
            bkk=np.ascontiguousarray((bk + bkb)[cs], f),
        ))

    in_maps = []
    for core in range(8):
        b, hg = core // 2, core % 2
        xq_t, xk_t, xkb_t, xv_t, vm = batch_data[b]
        m = dict(xq=xq_t, xk=xk_t, xkb=xkb_t, xv=xv_t, vm=vm)
        m.update(group_data[hg])
        in_maps.append(m)
    return in_maps, npad


def kernel(q, k, v, k_b, mask, Wq, bq, Wk, bk, Wv, bv, Wkb, bkb, Wo, bo):
    from concourse.bass_utils import run_bass_kernel_spmd

    q, k, v, k_b = (np.asarray(x, np.float32) for x in (q, k, v, k_b))
    mask = np.asarray(mask)
    in_maps, npad = make_in_maps(q, k, v, k_b, mask, Wq, bq, Wk, bk, Wv, bv,
                                 Wkb, bkb, Wo, bo)
    nc = _get_nc(npad)
    res = run_bass_kernel_spmd(nc, in_maps, list(range(8))).results
    bo = np.asarray(bo, np.float32)
    out = np.empty((B, S, H), np.float32)
    for b in range(B):
        out[b] = res[2 * b]["out"] + res[2 * b + 1]["out"] + bo
    return out


# revision 14
# speedup vs baseline: 1.4962x; 1.1303x over previous
"""CrossMultiHeadedAttention Trainium2 kernel (bf16 fast path).

Problem: B=4, S=2048, H=512, NH=8 heads, D=64.
  qh = (q @ Wq + bq), kh = (k @ Wk + bk), kbh = (k_b @ Wkb + bkb), vh = (v @ Wv + bv)
  scores = qh @ (kh + kbh)^T / sqrt(D), masked where mask[key]==0, softmax over keys
  out = (softmax @ vh heads concat) @ Wo + bo

Sharding: 8 cores = 4 batches x 2 head-groups (4 heads each).  Tensor-parallel
on the projections (Wq/Wk/Wv/Wkb column-split, Wo row-split); each core emits a
partial [S, H] output; host sums the two head-group partials per batch + bo.

Host-side prep (data movement only): gather unmasked k/k_b/v rows (mask depends
only on the key index; masked keys contribute exactly 0 after softmax), pad to
a multiple of 128, pre-transpose activations to [H, *] layout, and cast to
bf16.  This removes all PE transposes from the device program and keeps the PE
on the unthrottled bf16 path (fp32r matmuls are power-throttled to ~50% duty).

Padded keys carry zero k'/v rows.  Instead of a -1e9 additive bias before exp,
the softmax-denominator ones-column appended to V holds the key-validity mask
(1 real / 0 pad) and the V rows are scaled by it: pads then contribute exactly
0 to both the numerator and denominator, so exp needs no bias at all and can
drain two score PSUM banks per activation instruction.

Device layouts (per core):
  scores^T [keys, queries] per head; softmax normalizer via the validity
  column appended to V (PV matmul emits [65, q]: row 64 = sum of exp over real
  keys).  O' is normalized by 1/l via a DRAM-broadcast + DVE multiply, then
  the output projection contracts the 4 heads' dims in PSUM.
"""

import math

import ml_dtypes
import numpy as np

import concourse.bass as bass
import concourse.tile as tile
from concourse import mybir

F32 = mybir.dt.float32
BF16 = mybir.dt.bfloat16
NPBF16 = ml_dtypes.bfloat16

B, S, H, NH = 4, 2048, 512, 8
D = H // NH          # 64
HG = 4               # heads per core
HS = HG * D          # 256, per-core projection width
QC = S // 512        # query chunks of 512


def build_nc(npad: int) -> bass.Bass:
    KT = npad // 128          # key tiles
    nc = bass.Bass(target_bir_lowering=False, debug=False)

    xq = nc.declare_dram_parameter("xq", [H, S], BF16, isOutput=False)[:]
    xk = nc.declare_dram_parameter("xk", [H, npad], BF16, isOutput=False)[:]
    xkb = nc.declare_dram_parameter("xkb", [H, npad], BF16, isOutput=False)[:]
    xv = nc.declare_dram_parameter("xv", [H, npad], BF16, isOutput=False)[:]
    vm = nc.declare_dram_parameter("vm", [npad], F32, isOutput=False)[:]
    wq = nc.declare_dram_parameter("wq", [H, HS], BF16, isOutput=False)[:]
    wk = nc.declare_dram_parameter("wk", [H, HS], BF16, isOutput=False)[:]
    wkb = nc.declare_dram_parameter("wkb", [H, HS], BF16, isOutput=False)[:]
    wv = nc.declare_dram_parameter("wv", [640, HS], BF16, isOutput=False)[:]
    wo = nc.declare_dram_parameter("wo", [HS, H], BF16, isOutput=False)[:]
    bq = nc.declare_dram_parameter("bq", [HS], F32, isOutput=False)[:]
    bkk = nc.declare_dram_parameter("bkk", [HS], F32, isOutput=False)[:]
    out = nc.declare_dram_parameter("out", [S, H], F32, isOutput=True)[:]
    lscratch = nc.dram_tensor("lscratch", [QC * HG, 512], F32)[:]

    Exp = mybir.ActivationFunctionType.Exp

    # key chunks for the K' projection / DMA staging: two small leading
    # chunks so the first matmuls start within a few us of launch
    kcw = []
    off = 0
    while off < npad:
        w = min(256 if off < 512 else 512, npad - off)
        kcw.append((off, w))
        off += w
    # kt tile pairs for two-bank exp drains
    groups = [list(range(g, min(g + 2, KT))) for g in range(0, KT, 2)]

    with tile.TileContext(nc) as tc:
        with (
            tc.tile_pool(name="const", bufs=1) as cpool,
            tc.tile_pool(name="persist", bufs=1) as ppool,
            tc.tile_pool(name="probs", bufs=4) as prpool,
            tc.tile_pool(name="norm", bufs=3) as nrpool,
            tc.tile_pool(name="outs", bufs=3) as outpool,
        ):
            # DMA emission order puts the K-projection's dependencies on the
            # first DMA queues so the PE can start within a few us.
            wk_sb = cpool.tile([128, 4, HS], BF16, tag="wk")
            wkb_sb = cpool.tile([128, 4, HS], BF16, tag="wkb")
            wq_sb = cpool.tile([128, 4, HS], BF16, tag="wq")
            wv_sb = cpool.tile([128, 5, HS], BF16, tag="wv")
            wo_sb = cpool.tile([128, 2, H], BF16, tag="wo")
            wk_r = wk.rearrange("(t p) n -> p t n", p=128)
            wkb_r = wkb.rearrange("(t p) n -> p t n", p=128)
            wq_r = wq.rearrange("(t p) n -> p t n", p=128)
            wv_r = wv.rearrange("(t p) n -> p t n", p=128)

            kx = ppool.tile([128, 4, npad], BF16, tag="kx")
            kbx = ppool.tile([128, 4, npad], BF16, tag="kbx")
            vx = ppool.tile([128, 4, npad], BF16, tag="vx")
            qx = ppool.tile([128, 4, S], BF16, tag="qx")
            xk_r = xk.rearrange("(t p) n -> p t n", p=128)
            xkb_r = xkb.rearrange("(t p) n -> p t n", p=128)
            xv_r = xv.rearrange("(t p) n -> p t n", p=128)
            xq_r = xq.rearrange("(t p) n -> p t n", p=128)

            for t in range(4):
                nc.sync.dma_start(wk_sb[:, t], wk_r[:, t])
                nc.sync.dma_start(wkb_sb[:, t], wkb_r[:, t])
            off0, w0 = kcw[0]
            for t in range(4):
                nc.sync.dma_start(kx[:, t, off0:off0 + w0], xk_r[:, t, off0:off0 + w0])
                nc.sync.dma_start(kbx[:, t, off0:off0 + w0], xkb_r[:, t, off0:off0 + w0])
            bq_sb = cpool.tile([128, 2], F32, tag="bq")
            nc.sync.dma_start(bq_sb[:], bq.rearrange("(t p) -> p t", p=128))
            bkk_sb = cpool.tile([128, 2], F32, tag="bkk")
            nc.sync.dma_start(bkk_sb[:], bkk.rearrange("(t p) -> p t", p=128))
            vm_sb = cpool.tile([128, KT], F32, tag="vm")
            nc.sync.dma_start(vm_sb[:], vm.rearrange("(t p) -> p t", p=128))
            ones_c = cpool.tile([1, 128], BF16, tag="ones")
            nc.gpsimd.memset(ones_c[:], 1.0)
            for (off, w) in kcw[1:]:
                for t in range(4):
                    nc.sync.dma_start(kx[:, t, off:off + w], xk_r[:, t, off:off + w])
                    nc.sync.dma_start(kbx[:, t, off:off + w], xkb_r[:, t, off:off + w])
            for t in range(5):
                nc.sync.dma_start(wv_sb[:, t], wv_r[:, t])
            for (off, w) in kcw:
                for t in range(4):
                    nc.sync.dma_start(vx[:, t, off:off + w], xv_r[:, t, off:off + w])
            for t in range(4):
                nc.sync.dma_start(wq_sb[:, t], wq_r[:, t])
            for c in range(QC):
                for t in range(4):
                    nc.sync.dma_start(
                        qx[:, t, c * 512:(c + 1) * 512],
                        xq_r[:, t, c * 512:(c + 1) * 512],
                    )
            nc.sync.dma_start(wo_sb[:], wo.rearrange("(t p) n -> p t n", p=128))

            qT = ppool.tile([128, 2, S], BF16, tag="qT")
            kT = ppool.tile([128, 2, npad], BF16, tag="kT")
            v_sb = ppool.tile([128, KT, HG, 65], BF16, tag="v")
            o_sb = ppool.tile([128, 2, S], BF16, tag="o")
            # validity column (softmax denominator counts real keys only)
            vm_col = vm_sb[:].rearrange("p (t o) -> p t o", o=1)
            for h in range(HG):
                nc.gpsimd.tensor_copy(v_sb[:, :, h, 64:65], vm_col)

            with (
                tc.tile_pool(name="ps_p", bufs=2, space="PSUM") as ps_p,
                tc.tile_pool(name="ps_s", bufs=2, space="PSUM") as ps_s,
                tc.tile_pool(name="ps_o", bufs=2, space="PSUM") as ps_o,
            ):
                # ---- K' = k@Wk + k_b@Wkb + (bk+bkb), transposed layout ----
                for (off, w) in kcw:
                    for hp in range(2):
                        psk = ps_p.tile([128, 512], F32, tag="ps_p")
                        for t in range(4):
                            nc.tensor.matmul(
                                psk[:, :w],
                                wk_sb[:, t, hp * 128:(hp + 1) * 128],
                                kx[:, t, off:off + w],
                                start=(t == 0), stop=False,
                            )
                        for t in range(4):
                            nc.tensor.matmul(
                                psk[:, :w],
                                wkb_sb[:, t, hp * 128:(hp + 1) * 128],
                                kbx[:, t, off:off + w],
                                start=False, stop=(t == 3),
                            )
                        nc.vector.tensor_scalar_add(
                            kT[:, hp, off:off + w], psk[:, :w], bkk_sb[:, hp:hp + 1]
                        )

                # ---- V rows (gated by validity so pads contribute 0) ----
                for kt in range(KT):
                    psv = ps_p.tile([128, 512], F32, tag="ps_p")
                    for t in range(4):
                        nc.tensor.matmul(
                            psv[:, :HS],
                            vx[:, t, kt * 128:(kt + 1) * 128],
                            wv_sb[:, t, :],
                            start=(t == 0), stop=False,
                        )
                    nc.tensor.matmul(
                        psv[:, :HS], ones_c[0:1, :], wv_sb[0:1, 4, :],
                        start=False, stop=True,
                    )
                    nc.vector.tensor_scalar_mul(
                        v_sb[:, kt, :, 0:64],
                        psv[:, :HS].rearrange("p (h d) -> p h d", h=HG),
                        vm_sb[:, kt:kt + 1],
                    )

                # ---- per query chunk: Q projection, attention, out proj.
                # Qproj(c+1) and outproj(c-1) are emitted between heads of
                # attention(c) as PE filler for the exp-latency gaps. ----
                def qproj(c, hp):
                    q0 = c * 512
                    psq = ps_p.tile([128, 512], F32, tag="ps_p")
                    for t in range(4):
                        nc.tensor.matmul(
                            psq[:],
                            wq_sb[:, t, hp * 128:(hp + 1) * 128],
                            qx[:, t, q0:q0 + 512],
                            start=(t == 0), stop=(t == 3),
                        )
                    nc.vector.tensor_scalar_add(
                        qT[:, hp, q0:q0 + 512], psq[:], bq_sb[:, hp:hp + 1]
                    )

                def outproj(tt):
                    psf = ps_p.tile([128, 512], F32, tag="ps_p")
                    for hp in range(2):
                        nc.tensor.matmul(
                            psf[:],
                            o_sb[:, hp, tt * 128:(tt + 1) * 128],
                            wo_sb[:, hp, :],
                            start=(hp == 0), stop=(hp == 1),
                        )
                    ob = outpool.tile([128, H], F32, tag="ob")
                    nc.vector.tensor_copy(ob[:], psf[:])
                    nc.sync.dma_start(out[tt * 128:tt * 128 + 64, :], ob[0:64, :])
                    nc.sync.dma_start(out[tt * 128 + 64:(tt + 1) * 128, :], ob[64:128, :])

                qproj(0, 0)
                qproj(0, 1)
                for c in range(QC):
                    q0 = c * 512
                    fillers = []
                    if c + 1 < QC:
                        fillers += [lambda hp=hp: qproj(c + 1, hp) for hp in range(2)]
                    if c > 0:
                        fillers += [
                            lambda s=s: [outproj((c - 1) * 4 + 2 * s + i) for i in range(2)]
                            for s in range(2)
                        ]

                    for h in range(HG):
                        hp, hd = h // 2, h % 2
                        dlo, dhi = hd * 64, (hd + 1) * 64
                        pso = ps_o.tile([65, 512], F32, tag="ps_o")
                        first_pv = [True]

                        def drain(pss, grp, last):
                            w = len(grp) * 512
                            p = prpool.tile([128, 1024], BF16, tag="p")
                            nc.scalar.activation(p[:, :w], pss[:, :w], Exp)
                            for j, kt in enumerate(grp):
                                nc.tensor.matmul(
                                    pso[:],
                                    v_sb[:, kt, h, :],
                                    p[:, j * 512:(j + 1) * 512],
                                    start=first_pv[0],
                                    stop=(last and j == len(grp) - 1),
                                )
                                first_pv[0] = False

                        prev = None
                        for grp in groups:
                            pss = ps_s.tile([128, 1024], F32, tag="ps_s")
                            for j, kt in enumerate(grp):
                                nc.tensor.matmul(
                                    pss[:, j * 512:(j + 1) * 512],
                                    kT[dlo:dhi, hp, kt * 128:(kt + 1) * 128],
                                    qT[dlo:dhi, hp, q0:q0 + 512],
                                    start=True, stop=True,
                                )
                            if prev is not None:
                                drain(*prev, last=False)
                            prev = (pss, grp)
                        drain(*prev, last=True)

                        # free pso quickly: stash unnormalized O (bf16), then
                        # 1/l -> DRAM broadcast -> in-place normalize, all off
                        # the PE critical path
                        nc.vector.tensor_copy(
                            o_sb[dlo:dhi, hp, q0:q0 + 512], pso[0:64, :]
                        )
                        linv = nrpool.tile([1, 512], F32, tag="linv")
                        nc.vector.reciprocal(linv[:], pso[64:65, :])
                        lrow = lscratch[c * HG + h:c * HG + h + 1, :]
                        nc.sync.dma_start(lrow, linv[:])
                        lbc = nrpool.tile([128, 512], F32, tag="lbc")
                        lsrc, _ = bass.broadcast_tensor_aps(lrow, lbc[dlo:dhi, :])
                        nc.sync.dma_start(lbc[dlo:dhi, :], lsrc)
                        nc.vector.tensor_mul(
                            o_sb[dlo:dhi, hp, q0:q0 + 512],
                            o_sb[dlo:dhi, hp, q0:q0 + 512], lbc[dlo:dhi, :],
                        )
                        if h < len(fillers):
                            fillers[h]()

                for tt in range((QC - 1) * 4, QC * 4):
                    outproj(tt)
    _split_matmul_waits(nc)
    return nc


def _split_matmul_waits(nc: bass.Bass):
    """Walrus's matmul (LDW+MM) and DMA lowerings only fit one sync
    wait, but Tile may attach several.  Move the extras onto same-queue NOPs
    inserted right before each offending instruction."""
    eng_map = {
        mybir.EngineType.PE: nc.tensor,
        mybir.EngineType.SP: nc.sync,
        mybir.EngineType.Activation: nc.scalar,
        mybir.EngineType.DVE: nc.vector,
        mybir.EngineType.Pool: nc.gpsimd,
    }
    f = nc.m.functions[0]
    blocks = list(f.blocks)
    endblk = blocks[-1]
    n_nops = 0
    for blk in blocks:
        insts = blk.instructions
        if not any(
                x.sync_info is not None and len(x.sync_info.on_wait) > 1
                for x in insts):
            continue
        new = []
        changed = False
        for inst in insts:
            si = inst.sync_info
            if (si is not None and len(si.on_wait) > 1
                    and inst.engine in eng_map):
                waits = list(si.on_wait)
                for w in waits[:-1]:
                    nop = eng_map[inst.engine].nop().ins
                    n_nops += 1
                    nop.sync_info = type(si)(on_wait=[w], on_update=[])
                    new.append(nop)
                inst.sync_info = type(si)(on_wait=[waits[-1]],
                                          on_update=list(si.on_update))
                changed = True
            new.append(inst)
        if changed:
            blk.instructions = new
    if n_nops:
        # the .nop() calls appended to the tail block; strip them.
        endblk.instructions = endblk.instructions[:-n_nops]


_NC_CACHE: dict[int, bass.Bass] = {}


def _get_nc(npad: int) -> bass.Bass:
    if npad not in _NC_CACHE:
        _NC_CACHE[npad] = build_nc(npad)
    return _NC_CACHE[npad]


def make_in_maps(q, k, v, k_b, mask, Wq, bq, Wk, bk, Wv, bv, Wkb, bkb, Wo, bo):
    """Host-side sharding: returns (in_maps for cores 0..7, npad)."""
    f = np.float32
    bf = NPBF16
    sels = [np.nonzero(mask[b])[0] for b in range(B)]
    nmax = max(len(s) for s in sels)
    npad = max(128, int(math.ceil(nmax / 128.0)) * 128)
    scale = f(1.0 / math.sqrt(D))

    batch_data = []
    for b in range(B):
        sel = sels[b]
        n = len(sel)
        xq_t = np.ascontiguousarray(q[b].T).astype(bf)
        xk_t = np.zeros((H, npad), bf)
        xkb_t = np.zeros((H, npad), bf)
        xv_t = np.zeros((H, npad), bf)
        xk_t[:, :n] = k[b][sel].T.astype(bf)
        xkb_t[:, :n] = k_b[b][sel].T.astype(bf)
        xv_t[:, :n] = v[b][sel].T.astype(bf)
        vm = np.zeros((npad,), f)
        vm[:n] = 1.0
        batch_data.append((xq_t, xk_t, xkb_t, xv_t, vm))

    group_data = []
    for hg in range(2):
        cs = slice(hg * HS, (hg + 1) * HS)
        wv_aug = np.zeros((640, HS), bf)
        wv_aug[:H] = Wv[:, cs].astype(bf)
        wv_aug[H] = bv[cs].astype(bf)
        group_data.append(dict(
            wq=np.ascontiguousarray(Wq[:, cs] * scale).astype(bf),
            wk=np.ascontiguousarray(Wk[:, cs]).astype(bf),
            wkb=np.ascontiguousarray(Wkb[:, cs]).astype(bf),
            wv=wv_aug,
            wo=np.ascontiguousarray(Wo[cs, :]).astype(bf),
            bq=np.ascontiguousarray(bq[cs] * scale, f),
            bkk=np.ascontiguousarray((bk + bkb)[cs], f),
        ))

    in_maps = []
    for core in range(8):
        b, hg = core // 2, core % 2
        xq_t, xk_t, xkb_t, xv_t, vm = batch_data[b]
        m = dict(xq=xq_t, xk=xk_t, xkb=xkb_t, xv=xv_t, vm=vm)
        m.update(group_data[hg])
        in_maps.append(m)
    return in_maps, npad


def kernel(q, k, v, k_b, mask, Wq, bq, Wk, bk, Wv, bv, Wkb, bkb, Wo, bo):
    from concourse.bass_utils import run_bass_kernel_spmd

    q, k, v, k_b = (np.asarray(x, np.float32) for x in (q, k, v, k_b))
    mask = np.asarray(mask)
    in_maps, npad = make_in_maps(q, k, v, k_b, mask, Wq, bq, Wk, bk, Wv, bv,
                                 Wkb, bkb, Wo, bo)
    nc = _get_nc(npad)
    res = run_bass_kernel_spmd(nc, in_maps, list(range(8))).results
    bo = np.asarray(bo, np.float32)
    out = np.empty((B, S, H), np.float32)
    for b in range(B):
        out[b] = res[2 * b]["out"] + res[2 * b + 1]["out"] + bo
    return out
